# revision 1
# baseline (speedup 1.0000x reference)
"""Trainium2 Bass kernel for nn_CheapChannelV1 (dense_cnn).

Strategy (per core, pure data-parallel over batch):
  - The three channel-shuffle + 1x1-conv stages are linear, so they fold on the
    host into ONE 128x128 matrix M and bias b_tot:  res3 = M @ s + b_tot, where
    s = [s0;s1;s2;s3] are the four depthwise-conv branch outputs.
  - Level-0 depthwise conv (full res) folds INTO the matmul: 9 tap matmuls
    (K=32) reading shifted views of a zero-padded x0 strip tile.
  - Levels 1-3: max-pool on DVE, 3x3 depthwise conv on DVE in a 4-row-block
    strip layout; nearest-upsample folds into broadcast (step-0) rhs APs of the
    group matmuls.
  - 12 accumulating K=32 matmuls per 512-px chunk, spread across the four PE
    row groups via tile_position for concurrency.
  - Epilogue: exact Gelu on ACT (bias folded in), multiply-by-x on GPSIMD.
"""

import numpy as np

H = W = 256
CH = 128
NB = 8        # bands
SB = 8        # image rows per row-block per band (band covers 4*SB rows)
_DT = None    # lazy: mybir.dt.float32


def _shuf_cols(A, groups=8):
    # Returns A' with A' @ s == A @ channel_shuffle(s)
    Cin = A.shape[1]
    idx = np.arange(Cin)
    perm = (idx % groups) * (Cin // groups) + idx // groups
    Ap = np.zeros_like(A)
    Ap[:, perm] = A
    return Ap


def fold_weights(w_dw, b_dw, w_f1, b_f1, w_f2, b_f2, w_f3, b_f3):
    f8 = np.float64
    A1 = _shuf_cols(w_f1.astype(f8))
    A2 = _shuf_cols(w_f2.astype(f8))
    A3 = _shuf_cols(w_f3.astype(f8))
    A2a, A2b = A2[:, :64], A2[:, 64:]
    A3a, A3b = A3[:, :96], A3[:, 96:]
    M = np.zeros((128, 128), f8)
    M[:, 0:64] = A3a @ A2a @ A1
    M[:, 64:96] = A3a @ A2b
    M[:, 96:128] = A3b
    b_tot = A3a @ (A2a @ b_f1.astype(f8) + b_f2.astype(f8)) + b_f3.astype(f8)
    for g in range(4):
        b_tot = b_tot + M[:, 32 * g:32 * g + 32] @ b_dw[g].astype(f8)

    # W_all[p, t, o]: lhsT matrices, identical content per 32-partition group.
    W_all = np.zeros((128, 12, 128), np.float32)
    M0T = M[:, 0:32].T          # [32(c), 128(o)]
    w0 = w_dw[0].reshape(32, 9).astype(f8)
    for gp in range(4):
        rows = slice(32 * gp, 32 * gp + 32)
        for j in range(9):
            W_all[rows, j, :] = (M0T * w0[:, j:j + 1]).astype(np.float32)
        W_all[rows, 9, :] = M[:, 32:64].T.astype(np.float32)
        W_all[rows, 10, :] = M[:, 64:96].T.astype(np.float32)
        W_all[rows, 11, :] = M[:, 96:128].T.astype(np.float32)

    wdwp = np.zeros((128, 3, 9), np.float32)
    for g in (1, 2, 3):
        wdwp[:, g - 1, :] = np.tile(w_dw[g].reshape(32, 9), (4, 1)).astype(np.float32)

    return W_all, b_tot.astype(np.float32).reshape(128, 1), wdwp


def numpy_model(x, W_all, b_tot, wdwp):
    """Reference model of what the bass kernel computes (for one sample)."""
    from scipy.special import erf
    C, Hh, Ww = x.shape
    # level 0 conv via folded taps
    x0p = np.pad(x[:32], ((0, 0), (1, 1), (1, 1)))
    res3 = np.zeros((128, Hh, Ww), np.float32)
    for j in range(9):
        dy, dx = j // 3, j % 3
        lhsT = W_all[0:32, j, :]  # [32, 128]
        sh = x0p[:, dy:dy + Hh, dx:dx + Ww].reshape(32, -1)
        res3 += (lhsT.T @ sh).reshape(128, Hh, Ww)
    # pooled levels
    for g, k in ((1, 2), (2, 4), (3, 8)):
        xg = x[32 * g:32 * g + 32]
        p = xg.reshape(32, Hh // k, k, Ww // k, k).max(axis=(2, 4))
        pp = np.pad(p, ((0, 0), (1, 1), (1, 1)))
        conv = np.zeros_like(p)
        for j in range(9):
            dy, dx = j // 3, j % 3
            conv += wdwp[0:32, g - 1, j][:, None, None] * \
                pp[:, dy:dy + Hh // k, dx:dx + Ww // k]
        up = np.repeat(np.repeat(conv, k, axis=1), k, axis=2)
        lhsT = W_all[0:32, 8 + g, :]
        res3 += np.einsum('co,chw->ohw', lhsT, up)
    res3 = res3 + b_tot.reshape(128, 1, 1)
    g = 0.5 * res3 * (1.0 + erf(res3 / np.sqrt(2.0)))
    return (g * x).astype(np.float32)


_PROGRAM_CACHE = {}


def build_program(act_func_name="Gelu"):
    key = act_func_name
    if key in _PROGRAM_CACHE:
        return _PROGRAM_CACHE[key]

    import concourse.bacc as bacc
    import concourse.tile as tile
    import concourse.mybir as mybir

    f32 = mybir.dt.float32
    AOT = mybir.AluOpType
    act_func = getattr(mybir.ActivationFunctionType, act_func_name)

    nc = bacc.Bacc("TRN2", target_bir_lowering=False, debug=False)
    x_d = nc.dram_tensor("x", [CH, H, W], f32, kind="ExternalInput")
    wall_d = nc.dram_tensor("wall", [128, 12, 128], f32, kind="ExternalInput")
    btot_d = nc.dram_tensor("btot", [128, 1], f32, kind="ExternalInput")
    wdwp_d = nc.dram_tensor("wdwp", [128, 3, 9], f32, kind="ExternalInput")
    out_d = nc.dram_tensor("out", [CH, H, W], f32, kind="ExternalOutput")

    with tile.TileContext(nc) as tc:
        with tc.tile_pool(name="persist", bufs=1) as pers, \
             tc.tile_pool(name="xband", bufs=2) as xpool, \
             tc.tile_pool(name="x0strip", bufs=2) as x0pool, \
             tc.tile_pool(name="ptmp", bufs=1) as ptmp, \
             tc.tile_pool(name="convb", bufs=2) as cpool, \
             tc.tile_pool(name="psum", bufs=8, space="PSUM") as pspool, \
             tc.tile_pool(name="gout", bufs=4) as gpool, \
             tc.tile_pool(name="mout", bufs=4) as mpool:

            wall = pers.tile([128, 12, 128], f32)
            nc.sync.dma_start(wall[:], wall_d[:])
            btot = pers.tile([128, 1], f32)
            nc.sync.dma_start(btot[:], btot_d[:])
            wdwp = pers.tile([128, 3, 9], f32)
            nc.sync.dma_start(wdwp[:], wdwp_d[:])

            p1pad = pers.tile([128, 34, 130], f32)
            p2pad = pers.tile([128, 18, 66], f32)
            p3pad = pers.tile([128, 10, 34], f32)
            nc.vector.memset(p1pad[:], 0.0)
            nc.vector.memset(p2pad[:], 0.0)
            nc.vector.memset(p3pad[:], 0.0)

            # Top halos for pooled strips: strip rho's first conv row needs the
            # last pooled row of block rho-1, which only streams in at band 7.
            # Pool it up-front from a redundant load of the 8 image rows
            # preceding each block (r=1,2,3).
            xh = xpool.tile([128, 3, 8, 256], f32, tag="xband")
            for r in (1, 2, 3):
                nc.sync.dma_start(xh[:, r - 1], x_d[:, 64 * r - 8:64 * r, :])
            hhp1 = ptmp.tile([128, 3, 8, 128], f32, tag="hp1")
            nc.vector.tensor_tensor(
                hhp1[:], xh[:, :, :, 0::2], xh[:, :, :, 1::2], AOT.max)
            hvp1 = ptmp.tile([128, 3, 4, 128], f32, tag="vp1")
            nc.vector.tensor_tensor(
                hvp1[:], hhp1[:, :, 0::2, :], hhp1[:, :, 1::2, :], AOT.max)
            hhp2 = ptmp.tile([128, 3, 4, 64], f32, tag="hp2")
            nc.vector.tensor_tensor(
                hhp2[:], hvp1[:, :, :, 0::2], hvp1[:, :, :, 1::2], AOT.max)
            hvp2 = ptmp.tile([128, 3, 2, 64], f32, tag="vp2")
            nc.vector.tensor_tensor(
                hvp2[:], hhp2[:, :, 0::2, :], hhp2[:, :, 1::2, :], AOT.max)
            hhp3 = ptmp.tile([128, 3, 2, 32], f32, tag="hp3")
            nc.vector.tensor_tensor(
                hhp3[:], hvp2[:, :, :, 0::2], hvp2[:, :, :, 1::2], AOT.max)
            hvp3 = ptmp.tile([128, 3, 1, 32], f32, tag="vp3")
            nc.vector.tensor_tensor(
                hvp3[:], hhp3[:, :, 0::2, :], hhp3[:, :, 1::2, :], AOT.max)
            for r in (1, 2, 3):
                g = r * 32
                nc.sync.dma_start(p1pad[g:g + 32, 0:1, 1:129],
                                  hvp1[32:64, r - 1, 3:4, :])
                nc.sync.dma_start(p2pad[g:g + 32, 0:1, 1:65],
                                  hvp2[64:96, r - 1, 1:2, :])
                nc.sync.dma_start(p3pad[g:g + 32, 0:1, 1:33],
                                  hvp3[96:128, r - 1, 0:1, :])

            xband_prev = x0_prev = None

            for it in range(NB + 1):
                # ---------------- load + pool phase (band b = it) ------------
                if it < NB:
                    b = it
                    xband = xpool.tile([128, 4, SB, 256], f32)
                    for r in range(4):
                        nc.sync.dma_start(
                            xband[:, r],
                            x_d[:, 64 * r + SB * b: 64 * r + SB * b + SB, :])

                    x0 = x0pool.tile([128, SB + 2, 258], f32)
                    # zero the left/right pad columns (0 and 257)
                    nc.vector.memset(x0[:, :, ::257], 0.0)
                    for r in range(4):
                        lo = 64 * r + SB * b - 1
                        hi = lo + SB + 2
                        dlo, dhi = 0, SB + 2
                        if lo < 0:
                            nc.vector.memset(x0[32 * r:32 * r + 32, 0, :], 0.0)
                            dlo, lo = 1, 0
                        if hi > 256:
                            nc.vector.memset(
                                x0[32 * r:32 * r + 32, SB + 1, :], 0.0)
                            dhi, hi = SB + 1, 256
                        nc.sync.dma_start(
                            x0[32 * r:32 * r + 32, dlo:dhi, 1:257],
                            x_d[0:32, lo:hi, :])

                    # hierarchical 2x2 max pooling (channels 32..127)
                    # full-128-partition ops (lanes for unused channel groups
                    # compute junk for free; only the real slices get read)
                    hp1 = ptmp.tile([128, 4, SB, 128], f32)
                    nc.vector.tensor_tensor(
                        hp1[:], xband[:, :, :, 0::2],
                        xband[:, :, :, 1::2], AOT.max)
                    vp1 = ptmp.tile([128, 4, SB // 2, 128], f32)
                    nc.vector.tensor_tensor(
                        vp1[:], hp1[:, :, 0::2, :],
                        hp1[:, :, 1::2, :], AOT.max)
                    hp2 = ptmp.tile([128, 4, SB // 2, 64], f32)
                    nc.vector.tensor_tensor(
                        hp2[:], vp1[:, :, :, 0::2],
                        vp1[:, :, :, 1::2], AOT.max)
                    vp2 = ptmp.tile([128, 4, SB // 4, 64], f32)
                    nc.vector.tensor_tensor(
                        vp2[:], hp2[:, :, 0::2, :],
                        hp2[:, :, 1::2, :], AOT.max)
                    hp3 = ptmp.tile([128, 4, SB // 4, 32], f32)
                    nc.vector.tensor_tensor(
                        hp3[:], vp2[:, :, :, 0::2],
                        vp2[:, :, :, 1::2], AOT.max)
                    vp3 = ptmp.tile([128, 4, SB // 8, 32], f32)
                    nc.vector.tensor_tensor(
                        vp3[:], hp3[:, :, 0::2, :],
                        hp3[:, :, 1::2, :], AOT.max)

                    # scatter into persistent padded strip buffers
                    for r in range(4):
                        # pooled strips: strip rho=r lives at group r
                        g0 = r * 32
                        nc.sync.dma_start(
                            p1pad[g0:g0 + 32, 4 * b + 1:4 * b + 5, 1:129],
                            vp1[32:64, r])
                        nc.sync.dma_start(
                            p2pad[g0:g0 + 32, 2 * b + 1:2 * b + 3, 1:65],
                            vp2[64:96, r])
                        nc.sync.dma_start(
                            p3pad[g0:g0 + 32, b + 1:b + 2, 1:33],
                            vp3[96:128, r])
                        if b == 0 and r > 0:   # bottom halos of strip r-1
                            gm = (r - 1) * 32
                            nc.sync.dma_start(
                                p1pad[gm:gm + 32, 33:34, 1:129],
                                vp1[32:64, r, 0:1, :])
                            nc.sync.dma_start(
                                p2pad[gm:gm + 32, 17:18, 1:65],
                                vp2[64:96, r, 0:1, :])
                            nc.sync.dma_start(
                                p3pad[gm:gm + 32, 9:10, 1:33],
                                vp3[96:128, r, 0:1, :])

                # ---------------- compute phase (band bb = it-1) -------------
                if it > 0:
                    bb = it - 1
                    # pooled convs for this band's window (all 4 strips at once)
                    conv1 = cpool.tile([128, 4, 128], f32)
                    conv2 = cpool.tile([128, 2, 64], f32)
                    conv3 = cpool.tile([128, 1, 32], f32)
                    for j in range(9):
                        dy, dx = j // 3, j % 3
                        a1 = p1pad[:, 4 * bb + dy:4 * bb + dy + 4, dx:dx + 128]
                        a2 = p2pad[:, 2 * bb + dy:2 * bb + dy + 2, dx:dx + 64]
                        a3 = p3pad[:, bb + dy:bb + dy + 1, dx:dx + 32]
                        if j == 0:
                            nc.vector.tensor_scalar_mul(
                                conv1[:], a1, wdwp[:, 0, 0:1])
                            nc.vector.tensor_scalar_mul(
                                conv2[:], a2, wdwp[:, 1, 0:1])
                            nc.vector.tensor_scalar_mul(
                                conv3[:], a3, wdwp[:, 2, 0:1])
                        else:
                            nc.vector.scalar_tensor_tensor(
                                conv1[:], a1, wdwp[:, 0, j:j + 1], conv1[:],
                                AOT.mult, AOT.add)
                            nc.vector.scalar_tensor_tensor(
                                conv2[:], a2, wdwp[:, 1, j:j + 1], conv2[:],
                                AOT.mult, AOT.add)
                            nc.vector.scalar_tensor_tensor(
                                conv3[:], a3, wdwp[:, 2, j:j + 1], conv3[:],
                                AOT.mult, AOT.add)

                    for i in range(SB // 2):
                        pss = [pspool.tile([128, 2, 256], f32, tag="pschunk",
                                           name=f"ps_{bb}_{i}_{r}")
                               for r in range(4)]
                        for t in range(12):
                            for r in range(4):
                                g0 = 32 * r
                                if t < 3:
                                    lhsT = wall[g0:g0 + 32, 9 + t, :]
                                    if t == 0:
                                        rhs = conv1[g0:g0 + 32, i, :] \
                                            .unsqueeze(1).unsqueeze(3) \
                                            .broadcast_to([32, 2, 128, 2])
                                    elif t == 1:
                                        rhs = conv2[g0:g0 + 32, i // 2, :] \
                                            .unsqueeze(1).unsqueeze(3) \
                                            .broadcast_to([32, 2, 64, 4])
                                    else:
                                        rhs = conv3[g0:g0 + 32, 0, :] \
                                            .unsqueeze(1).unsqueeze(3) \
                                            .broadcast_to([32, 2, 32, 8])
                                else:
                                    j = t - 3
                                    dy, dx = j // 3, j % 3
                                    lhsT = wall[g0:g0 + 32, j, :]
                                    rhs = x0_prev[g0:g0 + 32,
                                                  2 * i + dy:2 * i + dy + 2,
                                                  dx:dx + 256]
                                nc.tensor.matmul(
                                    pss[r][:], lhsT, rhs,
                                    start=(t == 0), stop=(t == 11),
                                    tile_position=(g0, 0))
                        for r in range(4):
                            gt = gpool.tile([128, 2, 256], f32, tag="gchunk")
                            nc.scalar.activation(
                                gt[:], pss[r][:], act_func, bias=btot[:, 0:1])
                            mt = mpool.tile([128, 2, 256], f32, tag="mchunk")
                            nc.gpsimd.tensor_mul(
                                mt[:], gt[:],
                                xband_prev[:, r, 2 * i:2 * i + 2, :])
                            h = 64 * r + SB * bb + 2 * i
                            nc.sync.dma_start(out_d[:, h:h + 2, :], mt[:])

                if it < NB:
                    xband_prev, x0_prev = xband, x0

    nc.compile()
    _PROGRAM_CACHE[key] = nc
    return nc


def kernel(x, w_dw, b_dw, w_f1, b_f1, w_f2, b_f2, w_f3, b_f3):
    from concourse.bass_utils import run_bass_kernel_spmd

    x = np.asarray(x)
    B = x.shape[0]
    W_all, b_tot, wdwp = fold_weights(
        np.asarray(w_dw), np.asarray(b_dw), np.asarray(w_f1), np.asarray(b_f1),
        np.asarray(w_f2), np.asarray(b_f2), np.asarray(w_f3), np.asarray(b_f3))

    nc = build_program("Gelu")
    in_maps = [{"x": np.ascontiguousarray(x[i], dtype=np.float32),
                "wall": W_all, "btot": b_tot, "wdwp": wdwp}
               for i in range(B)]
    res = run_bass_kernel_spmd(nc, in_maps, list(range(B)))
    out = np.stack([res.results[i]["out"] for i in range(B)], axis=0)
    return out.astype(np.float32)



# revision 3
# speedup vs baseline: 1.6082x; 1.6082x over previous
"""Trainium2 Bass kernel for nn_CheapChannelV1 (dense_cnn).

Strategy (per core, pure data-parallel over batch):
  - The three channel-shuffle + 1x1-conv stages are linear, so they fold on the
    host into ONE 128x128 matrix M and bias b_tot:  res3 = M @ s + b_tot, where
    s = [s0;s1;s2;s3] are the four depthwise-conv branch outputs.
  - Level-0 depthwise conv (full res) folds INTO the matmul: 9 tap matmuls
    (K=32) reading shifted views of a zero-padded x0 strip tile.
  - Levels 1-3: max-pool on DVE, 3x3 depthwise conv on DVE in a 4-row-block
    strip layout; nearest-upsample folds into broadcast (step-0) rhs APs of the
    group matmuls.
  - 12 accumulating K=32 matmuls per 512-px chunk, spread across the four PE
    row groups via tile_position for concurrency.
  - Whole datapath in bf16 (PSUM + conv accumulation in fp32): fp32 matmuls
    are LDWEIGHTS-bound (no FWL) and 2-pass; bf16 is ~4x on the PE and halves
    DMA traffic and DVE time.
  - Epilogue: exact Gelu on ACT (bias folded in), multiply-by-x on GPSIMD.
"""

import numpy as np

H = W = 256
CH = 128
NB = 8        # bands
SB = 8        # image rows per row-block per band (band covers 4*SB rows)


def _shuf_cols(A, groups=8):
    # Returns A' with A' @ s == A @ channel_shuffle(s)
    Cin = A.shape[1]
    idx = np.arange(Cin)
    perm = (idx % groups) * (Cin // groups) + idx // groups
    Ap = np.zeros_like(A)
    Ap[:, perm] = A
    return Ap


def fold_weights(w_dw, b_dw, w_f1, b_f1, w_f2, b_f2, w_f3, b_f3):
    f8 = np.float64
    A1 = _shuf_cols(w_f1.astype(f8))
    A2 = _shuf_cols(w_f2.astype(f8))
    A3 = _shuf_cols(w_f3.astype(f8))
    A2a, A2b = A2[:, :64], A2[:, 64:]
    A3a, A3b = A3[:, :96], A3[:, 96:]
    M = np.zeros((128, 128), f8)
    M[:, 0:64] = A3a @ A2a @ A1
    M[:, 64:96] = A3a @ A2b
    M[:, 96:128] = A3b
    b_tot = A3a @ (A2a @ b_f1.astype(f8) + b_f2.astype(f8)) + b_f3.astype(f8)
    for g in range(4):
        b_tot = b_tot + M[:, 32 * g:32 * g + 32] @ b_dw[g].astype(f8)

    # W_all[p, t, o]: lhsT matrices, identical content per 32-partition group.
    W_all = np.zeros((128, 12, 128), np.float32)
    M0T = M[:, 0:32].T          # [32(c), 128(o)]
    w0 = w_dw[0].reshape(32, 9).astype(f8)
    for gp in range(4):
        rows = slice(32 * gp, 32 * gp + 32)
        for j in range(9):
            W_all[rows, j, :] = (M0T * w0[:, j:j + 1]).astype(np.float32)
        W_all[rows, 9, :] = M[:, 32:64].T.astype(np.float32)
        W_all[rows, 10, :] = M[:, 64:96].T.astype(np.float32)
        W_all[rows, 11, :] = M[:, 96:128].T.astype(np.float32)

    wdwp = np.zeros((128, 3, 9), np.float32)
    for g in (1, 2, 3):
        wdwp[:, g - 1, :] = np.tile(w_dw[g].reshape(32, 9), (4, 1)).astype(np.float32)

    return W_all, b_tot.astype(np.float32).reshape(128, 1), wdwp


_PROGRAM_CACHE = {}


def build_program(act_func_name="Gelu"):
    key = act_func_name
    if key in _PROGRAM_CACHE:
        return _PROGRAM_CACHE[key]

    import concourse.bacc as bacc
    import concourse.tile as tile
    import concourse.mybir as mybir

    f32 = mybir.dt.float32
    bf16 = mybir.dt.bfloat16
    AOT = mybir.AluOpType
    act_func = getattr(mybir.ActivationFunctionType, act_func_name)

    nc = bacc.Bacc("TRN2", target_bir_lowering=False, debug=False)
    x_d = nc.dram_tensor("x", [CH, H, W], bf16, kind="ExternalInput")
    wall_d = nc.dram_tensor("wall", [128, 12, 128], bf16, kind="ExternalInput")
    btot_d = nc.dram_tensor("btot", [128, 1], f32, kind="ExternalInput")
    wdwp_d = nc.dram_tensor("wdwp", [128, 3, 9], f32, kind="ExternalInput")
    out_d = nc.dram_tensor("out", [CH, H, W], bf16, kind="ExternalOutput")

    with tile.TileContext(nc) as tc:
        with tc.tile_pool(name="persist", bufs=1) as pers, \
             tc.tile_pool(name="xband", bufs=2) as xpool, \
             tc.tile_pool(name="x0strip", bufs=2) as x0pool, \
             tc.tile_pool(name="ptmp", bufs=1) as ptmp, \
             tc.tile_pool(name="convb", bufs=2) as cpool, \
             tc.tile_pool(name="psum", bufs=8, space="PSUM") as pspool, \
             tc.tile_pool(name="gout", bufs=4) as gpool, \
             tc.tile_pool(name="mout", bufs=4) as mpool:

            # --- persistent weights / strips -----------------------------
            wall = pers.tile([128, 12, 128], bf16)
            nc.sync.dma_start(wall[:], wall_d[:])
            btot = pers.tile([128, 1], f32)
            nc.sync.dma_start(btot[:], btot_d[:])
            wdwp = pers.tile([128, 3, 9], f32)
            nc.sync.dma_start(wdwp[:], wdwp_d[:])

            p1pad = pers.tile([128, 34, 130], bf16)
            p2pad = pers.tile([128, 18, 66], bf16)
            p3pad = pers.tile([128, 10, 34], bf16)
            nc.vector.memset(p1pad[:], 0.0)
            nc.vector.memset(p2pad[:], 0.0)
            nc.vector.memset(p3pad[:], 0.0)

            # Top halos for pooled strips: strip rho's first conv row needs the
            # last pooled row of block rho-1, which only streams in at band 7.
            # Pool it up-front from a redundant load of the 8 image rows
            # preceding each block (r=1,2,3).
            xh = xpool.tile([128, 3, 8, 256], bf16, tag="xband")
            for r in (1, 2, 3):
                nc.sync.dma_start(xh[:, r - 1], x_d[:, 64 * r - 8:64 * r, :])
            hhp1 = ptmp.tile([128, 3, 8, 128], bf16, tag="hp1")
            nc.vector.tensor_tensor(
                hhp1[:], xh[:, :, :, 0::2], xh[:, :, :, 1::2], AOT.max)
            hvp1 = ptmp.tile([128, 3, 4, 128], bf16, tag="vp1")
            nc.vector.tensor_tensor(
                hvp1[:], hhp1[:, :, 0::2, :], hhp1[:, :, 1::2, :], AOT.max)
            hhp2 = ptmp.tile([128, 3, 4, 64], bf16, tag="hp2")
            nc.vector.tensor_tensor(
                hhp2[:], hvp1[:, :, :, 0::2], hvp1[:, :, :, 1::2], AOT.max)
            hvp2 = ptmp.tile([128, 3, 2, 64], bf16, tag="vp2")
            nc.vector.tensor_tensor(
                hvp2[:], hhp2[:, :, 0::2, :], hhp2[:, :, 1::2, :], AOT.max)
            hhp3 = ptmp.tile([128, 3, 2, 32], bf16, tag="hp3")
            nc.vector.tensor_tensor(
                hhp3[:], hvp2[:, :, :, 0::2], hvp2[:, :, :, 1::2], AOT.max)
            hvp3 = ptmp.tile([128, 3, 1, 32], bf16, tag="vp3")
            nc.vector.tensor_tensor(
                hvp3[:], hhp3[:, :, 0::2, :], hhp3[:, :, 1::2, :], AOT.max)
            for r in (1, 2, 3):
                g = r * 32
                nc.sync.dma_start(p1pad[g:g + 32, 0:1, 1:129],
                                  hvp1[32:64, r - 1, 3:4, :])
                nc.sync.dma_start(p2pad[g:g + 32, 0:1, 1:65],
                                  hvp2[64:96, r - 1, 1:2, :])
                nc.sync.dma_start(p3pad[g:g + 32, 0:1, 1:33],
                                  hvp3[96:128, r - 1, 0:1, :])

            xband_prev = x0_prev = None

            for it in range(NB + 1):
                # ---------------- load + pool phase (band b = it) ------------
                if it < NB:
                    b = it
                    xband = xpool.tile([128, 4, SB, 256], bf16)
                    for r in range(4):
                        nc.sync.dma_start(
                            xband[:, r],
                            x_d[:, 64 * r + SB * b: 64 * r + SB * b + SB, :])

                    x0 = x0pool.tile([128, SB + 2, 258], bf16)
                    # zero the left/right pad columns (0 and 257)
                    nc.vector.memset(x0[:, :, ::257], 0.0)
                    for r in range(4):
                        lo = 64 * r + SB * b - 1
                        hi = lo + SB + 2
                        dlo, dhi = 0, SB + 2
                        if lo < 0:
                            nc.vector.memset(x0[32 * r:32 * r + 32, 0, :], 0.0)
                            dlo, lo = 1, 0
                        if hi > 256:
                            nc.vector.memset(
                                x0[32 * r:32 * r + 32, SB + 1, :], 0.0)
                            dhi, hi = SB + 1, 256
                        nc.sync.dma_start(
                            x0[32 * r:32 * r + 32, dlo:dhi, 1:257],
                            x_d[0:32, lo:hi, :])

                    # hierarchical 2x2 max pooling (channels 32..127)
                    # full-128-partition ops (lanes for unused channel groups
                    # compute junk for free; only the real slices get read)
                    hp1 = ptmp.tile([128, 4, SB, 128], bf16)
                    nc.vector.tensor_tensor(
                        hp1[:], xband[:, :, :, 0::2],
                        xband[:, :, :, 1::2], AOT.max)
                    vp1 = ptmp.tile([128, 4, SB // 2, 128], bf16)
                    nc.vector.tensor_tensor(
                        vp1[:], hp1[:, :, 0::2, :],
                        hp1[:, :, 1::2, :], AOT.max)
                    hp2 = ptmp.tile([128, 4, SB // 2, 64], bf16)
                    nc.vector.tensor_tensor(
                        hp2[:], vp1[:, :, :, 0::2],
                        vp1[:, :, :, 1::2], AOT.max)
                    vp2 = ptmp.tile([128, 4, SB // 4, 64], bf16)
                    nc.vector.tensor_tensor(
                        vp2[:], hp2[:, :, 0::2, :],
                        hp2[:, :, 1::2, :], AOT.max)
                    hp3 = ptmp.tile([128, 4, SB // 4, 32], bf16)
                    nc.vector.tensor_tensor(
                        hp3[:], vp2[:, :, :, 0::2],
                        vp2[:, :, :, 1::2], AOT.max)
                    vp3 = ptmp.tile([128, 4, SB // 8, 32], bf16)
                    nc.vector.tensor_tensor(
                        vp3[:], hp3[:, :, 0::2, :],
                        hp3[:, :, 1::2, :], AOT.max)

                    # scatter into persistent padded strip buffers
                    for r in range(4):
                        # pooled strips: strip rho=r lives at group r
                        g0 = r * 32
                        nc.sync.dma_start(
                            p1pad[g0:g0 + 32, 4 * b + 1:4 * b + 5, 1:129],
                            vp1[32:64, r])
                        nc.sync.dma_start(
                            p2pad[g0:g0 + 32, 2 * b + 1:2 * b + 3, 1:65],
                            vp2[64:96, r])
                        nc.sync.dma_start(
                            p3pad[g0:g0 + 32, b + 1:b + 2, 1:33],
                            vp3[96:128, r])
                        if b == 0 and r > 0:   # bottom halos of strip r-1
                            gm = (r - 1) * 32
                            nc.sync.dma_start(
                                p1pad[gm:gm + 32, 33:34, 1:129],
                                vp1[32:64, r, 0:1, :])
                            nc.sync.dma_start(
                                p2pad[gm:gm + 32, 17:18, 1:65],
                                vp2[64:96, r, 0:1, :])
                            nc.sync.dma_start(
                                p3pad[gm:gm + 32, 9:10, 1:33],
                                vp3[96:128, r, 0:1, :])

                # ---------------- compute phase (band bb = it-1) -------------
                if it > 0:
                    bb = it - 1
                    # pooled convs for this band's window (all 4 strips at
                    # once); accumulate in fp32, final tap casts to bf16 for
                    # the matmul rhs.
                    c1f = cpool.tile([128, 4, 128], f32, tag="c1f")
                    c2f = cpool.tile([128, 2, 64], f32, tag="c2f")
                    c3f = cpool.tile([128, 1, 32], f32, tag="c3f")
                    conv1 = cpool.tile([128, 4, 128], bf16, tag="c1b")
                    conv2 = cpool.tile([128, 2, 64], bf16, tag="c2b")
                    conv3 = cpool.tile([128, 1, 32], bf16, tag="c3b")
                    for j in range(9):
                        dy, dx = j // 3, j % 3
                        a1 = p1pad[:, 4 * bb + dy:4 * bb + dy + 4, dx:dx + 128]
                        a2 = p2pad[:, 2 * bb + dy:2 * bb + dy + 2, dx:dx + 64]
                        a3 = p3pad[:, bb + dy:bb + dy + 1, dx:dx + 32]
                        if j == 0:
                            nc.vector.tensor_scalar_mul(
                                c1f[:], a1, wdwp[:, 0, 0:1])
                            nc.vector.tensor_scalar_mul(
                                c2f[:], a2, wdwp[:, 1, 0:1])
                            nc.vector.tensor_scalar_mul(
                                c3f[:], a3, wdwp[:, 2, 0:1])
                        else:
                            o1, o2, o3 = ((conv1, conv2, conv3) if j == 8
                                          else (c1f, c2f, c3f))
                            nc.vector.scalar_tensor_tensor(
                                o1[:], a1, wdwp[:, 0, j:j + 1], c1f[:],
                                AOT.mult, AOT.add)
                            nc.vector.scalar_tensor_tensor(
                                o2[:], a2, wdwp[:, 1, j:j + 1], c2f[:],
                                AOT.mult, AOT.add)
                            nc.vector.scalar_tensor_tensor(
                                o3[:], a3, wdwp[:, 2, j:j + 1], c3f[:],
                                AOT.mult, AOT.add)

                    for i in range(SB // 2):
                        pss = [pspool.tile([128, 2, 256], f32, tag="pschunk",
                                           name=f"ps_{bb}_{i}_{r}")
                               for r in range(4)]
                        for t in range(12):
                            for r in range(4):
                                g0 = 32 * r
                                if t < 3:
                                    lhsT = wall[g0:g0 + 32, 9 + t, :]
                                    if t == 0:
                                        rhs = conv1[g0:g0 + 32, i, :] \
                                            .unsqueeze(1).unsqueeze(3) \
                                            .broadcast_to([32, 2, 128, 2])
                                    elif t == 1:
                                        rhs = conv2[g0:g0 + 32, i // 2, :] \
                                            .unsqueeze(1).unsqueeze(3) \
                                            .broadcast_to([32, 2, 64, 4])
                                    else:
                                        rhs = conv3[g0:g0 + 32, 0, :] \
                                            .unsqueeze(1).unsqueeze(3) \
                                            .broadcast_to([32, 2, 32, 8])
                                else:
                                    j = t - 3
                                    dy, dx = j // 3, j % 3
                                    lhsT = wall[g0:g0 + 32, j, :]
                                    rhs = x0_prev[g0:g0 + 32,
                                                  2 * i + dy:2 * i + dy + 2,
                                                  dx:dx + 256]
                                nc.tensor.matmul(
                                    pss[r][:], lhsT, rhs,
                                    start=(t == 0), stop=(t == 11),
                                    tile_position=(g0, 0))
                        for r in range(4):
                            gt = gpool.tile([128, 2, 256], bf16, tag="gchunk")
                            nc.scalar.activation(
                                gt[:], pss[r][:], act_func, bias=btot[:, 0:1])
                            mt = mpool.tile([128, 2, 256], bf16, tag="mchunk")
                            nc.gpsimd.tensor_mul(
                                mt[:], gt[:],
                                xband_prev[:, r, 2 * i:2 * i + 2, :])
                            h = 64 * r + SB * bb + 2 * i
                            nc.sync.dma_start(out_d[:, h:h + 2, :], mt[:])

                if it < NB:
                    xband_prev, x0_prev = xband, x0

    nc.compile()
    _PROGRAM_CACHE[key] = nc
    return nc


def make_in_maps(x, w_dw, b_dw, w_f1, b_f1, w_f2, b_f2, w_f3, b_f3):
    import ml_dtypes
    bf = ml_dtypes.bfloat16
    x = np.asarray(x)
    B = x.shape[0]
    W_all, b_tot, wdwp = fold_weights(
        np.asarray(w_dw), np.asarray(b_dw), np.asarray(w_f1), np.asarray(b_f1),
        np.asarray(w_f2), np.asarray(b_f2), np.asarray(w_f3), np.asarray(b_f3))
    wall_b = np.ascontiguousarray(W_all.astype(bf))
    in_maps = [{"x": np.ascontiguousarray(x[i].astype(bf)),
                "wall": wall_b, "btot": b_tot, "wdwp": wdwp}
               for i in range(B)]
    return in_maps


def kernel(x, w_dw, b_dw, w_f1, b_f1, w_f2, b_f2, w_f3, b_f3):
    from concourse.bass_utils import run_bass_kernel_spmd

    x = np.asarray(x)
    B = x.shape[0]
    in_maps = make_in_maps(x, w_dw, b_dw, w_f1, b_f1, w_f2, b_f2, w_f3, b_f3)
    nc = build_program("Gelu")
    res = run_bass_kernel_spmd(nc, in_maps, list(range(B)))
    out = np.stack([res.results[i]["out"] for i in range(B)], axis=0)
    return out.astype(np.float32)


# revision 7
# speedup vs baseline: 1.6663x; 1.0361x over previous
"""Trainium2 Bass kernel for nn_CheapChannelV1 (dense_cnn).

Strategy (per core, pure data-parallel over batch):
  - The three channel-shuffle + 1x1-conv stages are linear, so they fold on the
    host into ONE 128x128 matrix M and bias b_tot:  res3 = M @ s + b_tot, where
    s = [s0;s1;s2;s3] are the four depthwise-conv branch outputs.
  - Level-0 depthwise conv (full res) folds INTO the matmul: 9 tap matmuls
    (K=32) reading shifted views of a zero-padded x0 strip tile.
  - Levels 1-3: max-pool on DVE (reduce_max over innermost pairs = 4 elem/cyc,
    unit-stride TT = 2 elem/cyc), 3x3 depthwise conv on DVE (fp32 accum),
    nearest-upsample folds into broadcast rhs APs of the group matmuls.
  - 12 accumulating K=32 matmuls per 512-px chunk, spread across the four PE
    row groups via tile_position; chunk pairs share LDWEIGHTS.
  - Whole datapath bf16 (PSUM + conv accum fp32): fp32 matmuls are
    LDWEIGHTS-bound and 2-pass; bf16 is ~4x on the PE, 2x DMA and DVE.
  - Epilogue: exact Gelu on ACT (bias folded in), multiply-by-x split across
    DVE (block 0) and GPSIMD (blocks 1-3), 16-row batched output DMAs issued
    from the Scalar queue.
"""

import numpy as np

H = W = 256
CH = 128
NB = 4        # bands
SB = 16       # image rows per row-block per band (band covers 4*SB rows)


def _shuf_cols(A, groups=8):
    # Returns A' with A' @ s == A @ channel_shuffle(s)
    Cin = A.shape[1]
    idx = np.arange(Cin)
    perm = (idx % groups) * (Cin // groups) + idx // groups
    Ap = np.zeros_like(A)
    Ap[:, perm] = A
    return Ap


def fold_weights(w_dw, b_dw, w_f1, b_f1, w_f2, b_f2, w_f3, b_f3):
    f8 = np.float64
    A1 = _shuf_cols(w_f1.astype(f8))
    A2 = _shuf_cols(w_f2.astype(f8))
    A3 = _shuf_cols(w_f3.astype(f8))
    A2a, A2b = A2[:, :64], A2[:, 64:]
    A3a, A3b = A3[:, :96], A3[:, 96:]
    M = np.zeros((128, 128), f8)
    M[:, 0:64] = A3a @ A2a @ A1
    M[:, 64:96] = A3a @ A2b
    M[:, 96:128] = A3b
    b_tot = A3a @ (A2a @ b_f1.astype(f8) + b_f2.astype(f8)) + b_f3.astype(f8)
    for g in range(4):
        b_tot = b_tot + M[:, 32 * g:32 * g + 32] @ b_dw[g].astype(f8)

    # W_all[p, t, o]: lhsT matrices, identical content per 32-partition group.
    W_all = np.zeros((128, 12, 128), np.float32)
    M0T = M[:, 0:32].T          # [32(c), 128(o)]
    w0 = w_dw[0].reshape(32, 9).astype(f8)
    for gp in range(4):
        rows = slice(32 * gp, 32 * gp + 32)
        for j in range(9):
            W_all[rows, j, :] = (M0T * w0[:, j:j + 1]).astype(np.float32)
        W_all[rows, 9, :] = M[:, 32:64].T.astype(np.float32)
        W_all[rows, 10, :] = M[:, 64:96].T.astype(np.float32)
        W_all[rows, 11, :] = M[:, 96:128].T.astype(np.float32)

    wdwp = np.zeros((128, 3, 9), np.float32)
    for g in (1, 2, 3):
        wdwp[:, g - 1, :] = np.tile(w_dw[g].reshape(32, 9), (4, 1)).astype(np.float32)

    return W_all, b_tot.astype(np.float32).reshape(128, 1), wdwp


_PROGRAM_CACHE = {}


def build_program(act_func_name="Gelu"):
    key = act_func_name
    if key in _PROGRAM_CACHE:
        return _PROGRAM_CACHE[key]

    import concourse.bacc as bacc
    import concourse.tile as tile
    import concourse.mybir as mybir

    f32 = mybir.dt.float32
    bf16 = mybir.dt.bfloat16
    AOT = mybir.AluOpType
    AXL = mybir.AxisListType
    act_func = getattr(mybir.ActivationFunctionType, act_func_name)

    nc = bacc.Bacc("TRN2", target_bir_lowering=False, debug=False)
    x_d = nc.dram_tensor("x", [CH, H, W], bf16, kind="ExternalInput")
    wall_d = nc.dram_tensor("wall", [128, 12, 128], bf16, kind="ExternalInput")
    btot_d = nc.dram_tensor("btot", [128, 1], f32, kind="ExternalInput")
    wdwp_d = nc.dram_tensor("wdwp", [128, 3, 9], f32, kind="ExternalInput")
    out_d = nc.dram_tensor("out", [CH, H, W], bf16, kind="ExternalOutput")

    # x viewed as [128, block r, row-in-block, col]
    x_blk = x_d[:].rearrange("p (r hh) w -> p r hh w", r=4)

    with tile.TileContext(nc) as tc:
        with tc.tile_pool(name="persist", bufs=1) as pers, \
             tc.tile_pool(name="xband", bufs=2) as xpool, \
             tc.tile_pool(name="x0strip", bufs=2) as x0pool, \
             tc.tile_pool(name="ptmp", bufs=1) as ptmp, \
             tc.tile_pool(name="convb", bufs=2) as cpool, \
             tc.tile_pool(name="psum", bufs=8, space="PSUM") as pspool, \
             tc.tile_pool(name="gout", bufs=4) as gpool, \
             tc.tile_pool(name="mout", bufs=4) as mpool:

            # --- persistent weights / strips -----------------------------
            wall = pers.tile([128, 12, 128], bf16)
            nc.sync.dma_start(wall[:], wall_d[:])
            btot = pers.tile([128, 1], f32)
            nc.sync.dma_start(btot[:], btot_d[:])
            wdwp = pers.tile([128, 3, 9], f32)
            nc.sync.dma_start(wdwp[:], wdwp_d[:])

            p1pad = pers.tile([128, 34, 130], bf16)
            p2pad = pers.tile([128, 18, 66], bf16)
            p3pad = pers.tile([128, 10, 34], bf16)
            nc.vector.memset(p1pad[:], 0.0)
            nc.vector.memset(p2pad[:], 0.0)
            nc.vector.memset(p3pad[:], 0.0)

            # Top halos for pooled strips: strip rho's first conv row needs
            # the last pooled row of block rho-1, which only streams in at the
            # last band. Pool it up-front from a redundant load of the 8 image
            # rows preceding each block (r=1,2,3).
            xh = ptmp.tile([128, 3, 8, 256], bf16, tag="xhalo")
            nc.sync.dma_start(xh[:], x_blk[:, 0:3, 56:64, :])
            hh1 = ptmp.tile([128, 24, 128], bf16, tag="hp1")
            nc.vector.tensor_reduce(
                hh1[:], xh[:].rearrange("p r h (w two) -> p (r h) w two", two=2),
                AXL.X, AOT.max)
            hv1 = ptmp.tile([128, 3, 4, 128], bf16, tag="vp1")
            h1v = hh1[:].rearrange("p (r h) w -> p r h w", r=3)
            nc.vector.tensor_tensor(
                hv1[:], h1v[:, :, 0::2, :], h1v[:, :, 1::2, :], AOT.max)
            hh2 = ptmp.tile([128, 12, 64], bf16, tag="hp2")
            nc.vector.tensor_reduce(
                hh2[:], hv1[:].rearrange("p r h (w two) -> p (r h) w two", two=2),
                AXL.X, AOT.max)
            hv2 = ptmp.tile([128, 3, 2, 64], bf16, tag="vp2")
            h2v = hh2[:].rearrange("p (r h) w -> p r h w", r=3)
            nc.vector.tensor_tensor(
                hv2[:], h2v[:, :, 0::2, :], h2v[:, :, 1::2, :], AOT.max)
            hh3 = ptmp.tile([128, 6, 32], bf16, tag="hp3")
            nc.vector.tensor_reduce(
                hh3[:], hv2[:].rearrange("p r h (w two) -> p (r h) w two", two=2),
                AXL.X, AOT.max)
            hv3 = ptmp.tile([128, 3, 1, 32], bf16, tag="vp3")
            h3v = hh3[:].rearrange("p (r h) w -> p r h w", r=3)
            nc.vector.tensor_tensor(
                hv3[:], h3v[:, :, 0::2, :], h3v[:, :, 1::2, :], AOT.max)
            for r in (1, 2, 3):
                g = r * 32
                nc.sync.dma_start(p1pad[g:g + 32, 0:1, 1:129],
                                  hv1[32:64, r - 1, 3:4, :])
                nc.sync.dma_start(p2pad[g:g + 32, 0:1, 1:65],
                                  hv2[64:96, r - 1, 1:2, :])
                nc.sync.dma_start(p3pad[g:g + 32, 0:1, 1:33],
                                  hv3[96:128, r - 1, 0:1, :])

            xband_prev = x0_prev = None

            for it in range(NB + 1):
                # ---------------- load + pool phase (band b = it) ------------
                if it < NB:
                    b = it
                    xband = xpool.tile([128, 4, SB, 256], bf16)
                    nc.sync.dma_start(
                        xband[:], x_blk[:, :, SB * b: SB * b + SB, :])

                    x0 = x0pool.tile([128, SB + 2, 258], bf16)
                    # zero the left/right pad columns (0 and 257)
                    nc.vector.memset(x0[:, :, ::257], 0.0)
                    for r in range(4):
                        lo = 64 * r + SB * b - 1
                        hi = lo + SB + 2
                        dlo, dhi = 0, SB + 2
                        if lo < 0:
                            nc.vector.memset(x0[32 * r:32 * r + 32, 0, :], 0.0)
                            dlo, lo = 1, 0
                        if hi > 256:
                            nc.vector.memset(
                                x0[32 * r:32 * r + 32, SB + 1, :], 0.0)
                            dhi, hi = SB + 1, 256
                        nc.sync.dma_start(
                            x0[32 * r:32 * r + 32, dlo:dhi, 1:257],
                            x_d[0:32, lo:hi, :])

                    # hierarchical 2x2 max pooling (channels 32..127).
                    # Horizontal steps: reduce_max over innermost pairs
                    # (unit stride, 4 elem/cyc); vertical steps: unit-stride
                    # TT max (2 elem/cyc). Full-128-partition ops (lanes for
                    # unused channel groups compute junk for free).
                    hp1 = ptmp.tile([128, 64, 128], bf16, tag="hp1")
                    nc.vector.tensor_reduce(
                        hp1[:],
                        xband[:].rearrange(
                            "p r h (w two) -> p (r h) w two", two=2),
                        AXL.X, AOT.max)
                    vp1 = ptmp.tile([128, 4, 8, 128], bf16, tag="vp1")
                    p1v = hp1[:].rearrange("p (r h) w -> p r h w", r=4)
                    nc.vector.tensor_tensor(
                        vp1[:], p1v[:, :, 0::2, :], p1v[:, :, 1::2, :],
                        AOT.max)
                    hp2 = ptmp.tile([128, 32, 64], bf16, tag="hp2")
                    nc.vector.tensor_reduce(
                        hp2[:],
                        vp1[:].rearrange(
                            "p r h (w two) -> p (r h) w two", two=2),
                        AXL.X, AOT.max)
                    vp2 = ptmp.tile([128, 4, 4, 64], bf16, tag="vp2")
                    p2v = hp2[:].rearrange("p (r h) w -> p r h w", r=4)
                    nc.vector.tensor_tensor(
                        vp2[:], p2v[:, :, 0::2, :], p2v[:, :, 1::2, :],
                        AOT.max)
                    hp3 = ptmp.tile([128, 16, 32], bf16, tag="hp3")
                    nc.vector.tensor_reduce(
                        hp3[:],
                        vp2[:].rearrange(
                            "p r h (w two) -> p (r h) w two", two=2),
                        AXL.X, AOT.max)
                    vp3 = ptmp.tile([128, 4, 2, 32], bf16, tag="vp3")
                    p3v = hp3[:].rearrange("p (r h) w -> p r h w", r=4)
                    nc.vector.tensor_tensor(
                        vp3[:], p3v[:, :, 0::2, :], p3v[:, :, 1::2, :],
                        AOT.max)

                    # scatter into persistent padded strip buffers
                    for r in range(4):
                        # pooled strips: strip rho=r lives at group r
                        g0 = r * 32
                        nc.sync.dma_start(
                            p1pad[g0:g0 + 32, 8 * b + 1:8 * b + 9, 1:129],
                            vp1[32:64, r])
                        nc.sync.dma_start(
                            p2pad[g0:g0 + 32, 4 * b + 1:4 * b + 5, 1:65],
                            vp2[64:96, r])
                        nc.sync.dma_start(
                            p3pad[g0:g0 + 32, 2 * b + 1:2 * b + 3, 1:33],
                            vp3[96:128, r])
                        if b == 0 and r > 0:   # bottom halos of strip r-1
                            gm = (r - 1) * 32
                            nc.sync.dma_start(
                                p1pad[gm:gm + 32, 33:34, 1:129],
                                vp1[32:64, r, 0:1, :])
                            nc.sync.dma_start(
                                p2pad[gm:gm + 32, 17:18, 1:65],
                                vp2[64:96, r, 0:1, :])
                            nc.sync.dma_start(
                                p3pad[gm:gm + 32, 9:10, 1:33],
                                vp3[96:128, r, 0:1, :])

                # ---------------- compute phase (band bb = it-1) -------------
                if it > 0:
                    bb = it - 1
                    # pooled convs for this band's window (all 4 strips at
                    # once); fp32 accumulation, final tap casts to bf16 for
                    # the matmul rhs.
                    c1f = cpool.tile([128, 8, 128], f32, tag="c1f")
                    c2f = cpool.tile([128, 4, 64], f32, tag="c2f")
                    c3f = cpool.tile([128, 2, 32], f32, tag="c3f")
                    conv1 = cpool.tile([128, 8, 128], bf16, tag="c1b")
                    conv2 = cpool.tile([128, 4, 64], bf16, tag="c2b")
                    conv3 = cpool.tile([128, 2, 32], bf16, tag="c3b")
                    for j in range(9):
                        dy, dx = j // 3, j % 3
                        a1 = p1pad[:, 8 * bb + dy:8 * bb + dy + 8, dx:dx + 128]
                        a2 = p2pad[:, 4 * bb + dy:4 * bb + dy + 4, dx:dx + 64]
                        a3 = p3pad[:, 2 * bb + dy:2 * bb + dy + 2, dx:dx + 32]
                        if j == 0:
                            nc.vector.tensor_scalar_mul(
                                c1f[:], a1, wdwp[:, 0, 0:1])
                            nc.vector.tensor_scalar_mul(
                                c2f[:], a2, wdwp[:, 1, 0:1])
                            nc.vector.tensor_scalar_mul(
                                c3f[:], a3, wdwp[:, 2, 0:1])
                        else:
                            o1, o2, o3 = ((conv1, conv2, conv3) if j == 8
                                          else (c1f, c2f, c3f))
                            nc.vector.scalar_tensor_tensor(
                                o1[:], a1, wdwp[:, 0, j:j + 1], c1f[:],
                                AOT.mult, AOT.add)
                            nc.vector.scalar_tensor_tensor(
                                o2[:], a2, wdwp[:, 1, j:j + 1], c2f[:],
                                AOT.mult, AOT.add)
                            nc.vector.scalar_tensor_tensor(
                                o3[:], a3, wdwp[:, 2, j:j + 1], c3f[:],
                                AOT.mult, AOT.add)

                    mts = [mpool.tile([128, SB, 256], bf16, tag="mchunk",
                                      name=f"mt_{bb}_{r}")
                           for r in range(4)]
                    for pg in range(SB // 4):     # pairs of 2-row chunks
                        pss = [[pspool.tile([128, 2, 256], f32, tag="pschunk",
                                            name=f"ps_{bb}_{pg}_{r}_{ic}")
                                for ic in range(2)] for r in range(4)]
                        for t in range(12):
                            for r in range(4):
                                g0 = 32 * r
                                lhsT = wall[g0:g0 + 32,
                                            (9 + t) if t < 3 else t - 3, :]
                                for ic in range(2):
                                    i = 2 * pg + ic
                                    if t == 0:
                                        rhs = conv1[g0:g0 + 32, i, :] \
                                            .unsqueeze(1).unsqueeze(3) \
                                            .broadcast_to([32, 2, 128, 2])
                                    elif t == 1:
                                        rhs = conv2[g0:g0 + 32, i // 2, :] \
                                            .unsqueeze(1).unsqueeze(3) \
                                            .broadcast_to([32, 2, 64, 4])
                                    elif t == 2:
                                        rhs = conv3[g0:g0 + 32, i // 4, :] \
                                            .unsqueeze(1).unsqueeze(3) \
                                            .broadcast_to([32, 2, 32, 8])
                                    else:
                                        j = t - 3
                                        dy, dx = j // 3, j % 3
                                        rhs = x0_prev[g0:g0 + 32,
                                                      2 * i + dy:2 * i + dy + 2,
                                                      dx:dx + 256]
                                    nc.tensor.matmul(
                                        pss[r][ic][:], lhsT, rhs,
                                        start=(t == 0), stop=(t == 11),
                                        tile_position=(g0, 0))
                        for r in range(4):
                            for ic in range(2):
                                i = 2 * pg + ic
                                gt = gpool.tile([128, 2, 256], bf16,
                                                tag="gchunk")
                                nc.scalar.activation(
                                    gt[:], pss[r][ic][:], act_func,
                                    bias=btot[:, 0:1])
                                xs = xband_prev[:, r, 2 * i:2 * i + 2, :]
                                ms = mts[r][:, 2 * i:2 * i + 2, :]
                                if r == 0:
                                    nc.vector.tensor_tensor(
                                        ms, gt[:], xs, AOT.mult)
                                else:
                                    nc.gpsimd.tensor_mul(ms, gt[:], xs)
                    for r in range(4):
                        nc.scalar.dma_start(
                            out_d[:, 64 * r + SB * bb: 64 * r + SB * bb + SB,
                                  :],
                            mts[r][:])

                if it < NB:
                    xband_prev, x0_prev = xband, x0

    nc.compile()
    _PROGRAM_CACHE[key] = nc
    return nc


def make_in_maps(x, w_dw, b_dw, w_f1, b_f1, w_f2, b_f2, w_f3, b_f3):
    import ml_dtypes
    bf = ml_dtypes.bfloat16
    x = np.asarray(x)
    B = x.shape[0]
    W_all, b_tot, wdwp = fold_weights(
        np.asarray(w_dw), np.asarray(b_dw), np.asarray(w_f1), np.asarray(b_f1),
        np.asarray(w_f2), np.asarray(b_f2), np.asarray(w_f3), np.asarray(b_f3))
    wall_b = np.ascontiguousarray(W_all.astype(bf))
    in_maps = [{"x": np.ascontiguousarray(x[i].astype(bf)),
                "wall": wall_b, "btot": b_tot, "wdwp": wdwp}
               for i in range(B)]
    return in_maps


def kernel(x, w_dw, b_dw, w_f1, b_f1, w_f2, b_f2, w_f3, b_f3):
    from concourse.bass_utils import run_bass_kernel_spmd

    x = np.asarray(x)
    B = x.shape[0]
    in_maps = make_in_maps(x, w_dw, b_dw, w_f1, b_f1, w_f2, b_f2, w_f3, b_f3)
    nc = build_program("Gelu")
    res = run_bass_kernel_spmd(nc, in_maps, list(range(B)))
    out = np.stack([res.results[i]["out"] for i in range(B)], axis=0)
    return out.astype(np.float32)


# revision 8
# speedup vs baseline: 2.2314x; 1.3391x over previous
"""Trainium2 Bass kernel for nn_CheapChannelV1 (dense_cnn).

Strategy (per core, pure data-parallel over batch):
  - The three channel-shuffle + 1x1-conv stages are linear, so they fold on the
    host into ONE 128x128 matrix M and bias b_tot:  res3 = M @ s + b_tot, where
    s = [s0;s1;s2;s3] are the four depthwise-conv branch outputs.
  - Level-0 depthwise conv (full res) folds INTO the matmul: 9 tap matmuls
    (K=32) reading shifted views of a zero-padded x0 strip tile.
  - Levels 1-3: max-pool on DVE (strided TT max), 3x3 depthwise conv on DVE
    (fp32 accum); nearest-upsample folds into broadcast rhs APs of the group
    matmuls.
  - 12 accumulating K=32 matmuls per 512-px chunk, spread across the four PE
    row groups via tile_position for quadrant concurrency.
  - Whole datapath bf16 (PSUM + conv accum fp32): fp32 matmuls are
    LDWEIGHTS-bound and 2-pass; bf16 is ~4x on the PE, 2x DMA.
  - Two-band-deep pipeline: band b is pooled at iteration b, its convs run at
    b+1, its matmuls/epilogue at b+2 — so the DVE phase of one band overlaps
    the PE/ACT/GPSIMD phase of the previous one.
  - Epilogue: exact Gelu on ACT (bias folded in), multiply-by-x on GPSIMD,
    16-row batched output DMAs issued from the Scalar queue.
"""

import numpy as np

H = W = 256
CH = 128
NB = 4        # bands
SB = 16       # image rows per row-block per band (band covers 4*SB rows)


def _shuf_cols(A, groups=8):
    # Returns A' with A' @ s == A @ channel_shuffle(s)
    Cin = A.shape[1]
    idx = np.arange(Cin)
    perm = (idx % groups) * (Cin // groups) + idx // groups
    Ap = np.zeros_like(A)
    Ap[:, perm] = A
    return Ap


def fold_weights(w_dw, b_dw, w_f1, b_f1, w_f2, b_f2, w_f3, b_f3):
    f8 = np.float64
    A1 = _shuf_cols(w_f1.astype(f8))
    A2 = _shuf_cols(w_f2.astype(f8))
    A3 = _shuf_cols(w_f3.astype(f8))
    A2a, A2b = A2[:, :64], A2[:, 64:]
    A3a, A3b = A3[:, :96], A3[:, 96:]
    M = np.zeros((128, 128), f8)
    M[:, 0:64] = A3a @ A2a @ A1
    M[:, 64:96] = A3a @ A2b
    M[:, 96:128] = A3b
    b_tot = A3a @ (A2a @ b_f1.astype(f8) + b_f2.astype(f8)) + b_f3.astype(f8)
    for g in range(4):
        b_tot = b_tot + M[:, 32 * g:32 * g + 32] @ b_dw[g].astype(f8)

    # W_all[p, t, o]: lhsT matrices, identical content per 32-partition group.
    W_all = np.zeros((128, 12, 128), np.float32)
    M0T = M[:, 0:32].T          # [32(c), 128(o)]
    w0 = w_dw[0].reshape(32, 9).astype(f8)
    for gp in range(4):
        rows = slice(32 * gp, 32 * gp + 32)
        for j in range(9):
            W_all[rows, j, :] = (M0T * w0[:, j:j + 1]).astype(np.float32)
        W_all[rows, 9, :] = M[:, 32:64].T.astype(np.float32)
        W_all[rows, 10, :] = M[:, 64:96].T.astype(np.float32)
        W_all[rows, 11, :] = M[:, 96:128].T.astype(np.float32)

    wdwp = np.zeros((128, 3, 9), np.float32)
    for g in (1, 2, 3):
        wdwp[:, g - 1, :] = np.tile(w_dw[g].reshape(32, 9), (4, 1)).astype(np.float32)

    return W_all, b_tot.astype(np.float32).reshape(128, 1), wdwp


_PROGRAM_CACHE = {}


def build_program(act_func_name="Gelu"):
    key = act_func_name
    if key in _PROGRAM_CACHE:
        return _PROGRAM_CACHE[key]

    import concourse.bacc as bacc
    import concourse.tile as tile
    import concourse.mybir as mybir

    f32 = mybir.dt.float32
    bf16 = mybir.dt.bfloat16
    AOT = mybir.AluOpType
    act_func = getattr(mybir.ActivationFunctionType, act_func_name)

    nc = bacc.Bacc("TRN2", target_bir_lowering=False, debug=False)
    x_d = nc.dram_tensor("x", [CH, H, W], bf16, kind="ExternalInput")
    wall_d = nc.dram_tensor("wall", [128, 12, 128], bf16, kind="ExternalInput")
    btot_d = nc.dram_tensor("btot", [128, 1], f32, kind="ExternalInput")
    wdwp_d = nc.dram_tensor("wdwp", [128, 3, 9], f32, kind="ExternalInput")
    out_d = nc.dram_tensor("out", [CH, H, W], bf16, kind="ExternalOutput")

    # x viewed as [128, block r, row-in-block, col]
    x_blk = x_d[:].rearrange("p (r hh) w -> p r hh w", r=4)

    with tile.TileContext(nc) as tc:
        with tc.tile_pool(name="persist", bufs=1) as pers, \
             tc.tile_pool(name="xband", bufs=3) as xpool, \
             tc.tile_pool(name="x0strip", bufs=2) as x0pool, \
             tc.tile_pool(name="ptmp", bufs=1) as ptmp, \
             tc.tile_pool(name="convb", bufs=2) as cpool, \
             tc.tile_pool(name="psum", bufs=8, space="PSUM") as pspool, \
             tc.tile_pool(name="gout", bufs=4) as gpool, \
             tc.tile_pool(name="mout", bufs=4) as mpool:

            # --- persistent weights / strips -----------------------------
            wall = pers.tile([128, 12, 128], bf16)
            nc.sync.dma_start(wall[:], wall_d[:])
            btot = pers.tile([128, 1], f32)
            nc.sync.dma_start(btot[:], btot_d[:])
            wdwp = pers.tile([128, 3, 9], f32)
            nc.sync.dma_start(wdwp[:], wdwp_d[:])

            p1pad = pers.tile([128, 34, 130], bf16)
            p2pad = pers.tile([128, 18, 66], bf16)
            p3pad = pers.tile([128, 10, 34], bf16)
            nc.vector.memset(p1pad[:], 0.0)
            nc.vector.memset(p2pad[:], 0.0)
            nc.vector.memset(p3pad[:], 0.0)

            # Top halos for pooled strips: strip rho's first conv row needs
            # the last pooled row of block rho-1, which only streams in at the
            # last band. Pool it up-front from a redundant load of the 8 image
            # rows preceding each block (r=1,2,3).
            xh = xpool.tile([128, 3, 8, 256], bf16, tag="xb")
            nc.sync.dma_start(xh[:], x_blk[:, 0:3, 56:64, :])
            hh1 = ptmp.tile([128, 3, 8, 128], bf16, tag="hp1")
            nc.vector.tensor_tensor(
                hh1[:], xh[:, :, :, 0::2], xh[:, :, :, 1::2], AOT.max)
            hv1 = ptmp.tile([128, 3, 4, 128], bf16, tag="vp1")
            nc.vector.tensor_tensor(
                hv1[:], hh1[:, :, 0::2, :], hh1[:, :, 1::2, :], AOT.max)
            hh2 = ptmp.tile([128, 3, 4, 64], bf16, tag="hp2")
            nc.vector.tensor_tensor(
                hh2[:], hv1[:, :, :, 0::2], hv1[:, :, :, 1::2], AOT.max)
            hv2 = ptmp.tile([128, 3, 2, 64], bf16, tag="vp2")
            nc.vector.tensor_tensor(
                hv2[:], hh2[:, :, 0::2, :], hh2[:, :, 1::2, :], AOT.max)
            hh3 = ptmp.tile([128, 3, 2, 32], bf16, tag="hp3")
            nc.vector.tensor_tensor(
                hh3[:], hv2[:, :, :, 0::2], hv2[:, :, :, 1::2], AOT.max)
            hv3 = ptmp.tile([128, 3, 1, 32], bf16, tag="vp3")
            nc.vector.tensor_tensor(
                hv3[:], hh3[:, :, 0::2, :], hh3[:, :, 1::2, :], AOT.max)
            for r in (1, 2, 3):
                g = r * 32
                nc.sync.dma_start(p1pad[g:g + 32, 0:1, 1:129],
                                  hv1[32:64, r - 1, 3:4, :])
                nc.sync.dma_start(p2pad[g:g + 32, 0:1, 1:65],
                                  hv2[64:96, r - 1, 1:2, :])
                nc.sync.dma_start(p3pad[g:g + 32, 0:1, 1:33],
                                  hv3[96:128, r - 1, 0:1, :])

            xbands, x0s, convs = {}, {}, {}

            for it in range(NB + 2):
                # ------------- load + pool phase (band b = it) ---------------
                if it < NB:
                    b = it
                    xband = xpool.tile([128, 4, SB, 256], bf16, tag="xb")
                    xbands[b] = xband
                    nc.sync.dma_start(
                        xband[:], x_blk[:, :, SB * b: SB * b + SB, :])

                    # hierarchical 2x2 max pooling (channels 32..127);
                    # full-128-partition ops (lanes for unused channel groups
                    # compute junk for free; only the real slices get read).
                    hp1 = ptmp.tile([128, 4, SB, 128], bf16, tag="hp1")
                    nc.vector.tensor_tensor(
                        hp1[:], xband[:, :, :, 0::2],
                        xband[:, :, :, 1::2], AOT.max)
                    vp1 = ptmp.tile([128, 4, SB // 2, 128], bf16, tag="vp1")
                    nc.vector.tensor_tensor(
                        vp1[:], hp1[:, :, 0::2, :],
                        hp1[:, :, 1::2, :], AOT.max)
                    hp2 = ptmp.tile([128, 4, SB // 2, 64], bf16, tag="hp2")
                    nc.vector.tensor_tensor(
                        hp2[:], vp1[:, :, :, 0::2],
                        vp1[:, :, :, 1::2], AOT.max)
                    vp2 = ptmp.tile([128, 4, SB // 4, 64], bf16, tag="vp2")
                    nc.vector.tensor_tensor(
                        vp2[:], hp2[:, :, 0::2, :],
                        hp2[:, :, 1::2, :], AOT.max)
                    hp3 = ptmp.tile([128, 4, SB // 4, 32], bf16, tag="hp3")
                    nc.vector.tensor_tensor(
                        hp3[:], vp2[:, :, :, 0::2],
                        vp2[:, :, :, 1::2], AOT.max)
                    vp3 = ptmp.tile([128, 4, SB // 8, 32], bf16, tag="vp3")
                    nc.vector.tensor_tensor(
                        vp3[:], hp3[:, :, 0::2, :],
                        hp3[:, :, 1::2, :], AOT.max)

                    # scatter into persistent padded strip buffers
                    for r in range(4):
                        g0 = r * 32
                        nc.sync.dma_start(
                            p1pad[g0:g0 + 32, 8 * b + 1:8 * b + 9, 1:129],
                            vp1[32:64, r])
                        nc.sync.dma_start(
                            p2pad[g0:g0 + 32, 4 * b + 1:4 * b + 5, 1:65],
                            vp2[64:96, r])
                        nc.sync.dma_start(
                            p3pad[g0:g0 + 32, 2 * b + 1:2 * b + 3, 1:33],
                            vp3[96:128, r])
                        if b == 0 and r > 0:   # bottom halos of strip r-1
                            gm = (r - 1) * 32
                            nc.sync.dma_start(
                                p1pad[gm:gm + 32, 33:34, 1:129],
                                vp1[32:64, r, 0:1, :])
                            nc.sync.dma_start(
                                p2pad[gm:gm + 32, 17:18, 1:65],
                                vp2[64:96, r, 0:1, :])
                            nc.sync.dma_start(
                                p3pad[gm:gm + 32, 9:10, 1:33],
                                vp3[96:128, r, 0:1, :])

                # ------------- x0 load + convs (band bc = it-1) --------------
                if 1 <= it <= NB:
                    bc = it - 1
                    x0 = x0pool.tile([128, SB + 2, 258], bf16)
                    x0s[bc] = x0
                    # zero the left/right pad columns (0 and 257)
                    nc.vector.memset(x0[:, :, ::257], 0.0)
                    for r in range(4):
                        lo = 64 * r + SB * bc - 1
                        hi = lo + SB + 2
                        dlo, dhi = 0, SB + 2
                        if lo < 0:
                            nc.vector.memset(x0[32 * r:32 * r + 32, 0, :], 0.0)
                            dlo, lo = 1, 0
                        if hi > 256:
                            nc.vector.memset(
                                x0[32 * r:32 * r + 32, SB + 1, :], 0.0)
                            dhi, hi = SB + 1, 256
                        nc.sync.dma_start(
                            x0[32 * r:32 * r + 32, dlo:dhi, 1:257],
                            x_d[0:32, lo:hi, :])

                    # pooled convs (all 4 strips at once); fp32 accumulation,
                    # final tap casts to bf16 for the matmul rhs.
                    c1f = ptmp.tile([128, 8, 128], f32, tag="c1f")
                    c2f = ptmp.tile([128, 4, 64], f32, tag="c2f")
                    c3f = ptmp.tile([128, 2, 32], f32, tag="c3f")
                    conv1 = cpool.tile([128, 8, 128], bf16, tag="c1b")
                    conv2 = cpool.tile([128, 4, 64], bf16, tag="c2b")
                    conv3 = cpool.tile([128, 2, 32], bf16, tag="c3b")
                    convs[bc] = (conv1, conv2, conv3)
                    for j in range(9):
                        dy, dx = j // 3, j % 3
                        a1 = p1pad[:, 8 * bc + dy:8 * bc + dy + 8, dx:dx + 128]
                        a2 = p2pad[:, 4 * bc + dy:4 * bc + dy + 4, dx:dx + 64]
                        a3 = p3pad[:, 2 * bc + dy:2 * bc + dy + 2, dx:dx + 32]
                        if j == 0:
                            nc.vector.tensor_scalar_mul(
                                c1f[:], a1, wdwp[:, 0, 0:1])
                            nc.vector.tensor_scalar_mul(
                                c2f[:], a2, wdwp[:, 1, 0:1])
                            nc.vector.tensor_scalar_mul(
                                c3f[:], a3, wdwp[:, 2, 0:1])
                        else:
                            o1, o2, o3 = ((conv1, conv2, conv3) if j == 8
                                          else (c1f, c2f, c3f))
                            nc.vector.scalar_tensor_tensor(
                                o1[:], a1, wdwp[:, 0, j:j + 1], c1f[:],
                                AOT.mult, AOT.add)
                            nc.vector.scalar_tensor_tensor(
                                o2[:], a2, wdwp[:, 1, j:j + 1], c2f[:],
                                AOT.mult, AOT.add)
                            nc.vector.scalar_tensor_tensor(
                                o3[:], a3, wdwp[:, 2, j:j + 1], c3f[:],
                                AOT.mult, AOT.add)

                # ------------- matmuls + epilogue (band bb = it-2) -----------
                if it >= 2:
                    bb = it - 2
                    conv1, conv2, conv3 = convs.pop(bb)
                    x0b = x0s.pop(bb)
                    xbb = xbands.pop(bb)
                    mts = [mpool.tile([128, SB, 256], bf16, tag="mchunk",
                                      name=f"mt_{bb}_{r}")
                           for r in range(4)]
                    for i in range(SB // 2):
                        pss = [pspool.tile([128, 2, 256], f32, tag="pschunk",
                                           name=f"ps_{bb}_{i}_{r}")
                               for r in range(4)]
                        for t in range(12):
                            for r in range(4):
                                g0 = 32 * r
                                if t < 3:
                                    lhsT = wall[g0:g0 + 32, 9 + t, :]
                                    if t == 0:
                                        rhs = conv1[g0:g0 + 32, i, :] \
                                            .unsqueeze(1).unsqueeze(3) \
                                            .broadcast_to([32, 2, 128, 2])
                                    elif t == 1:
                                        rhs = conv2[g0:g0 + 32, i // 2, :] \
                                            .unsqueeze(1).unsqueeze(3) \
                                            .broadcast_to([32, 2, 64, 4])
                                    else:
                                        rhs = conv3[g0:g0 + 32, i // 4, :] \
                                            .unsqueeze(1).unsqueeze(3) \
                                            .broadcast_to([32, 2, 32, 8])
                                else:
                                    j = t - 3
                                    dy, dx = j // 3, j % 3
                                    lhsT = wall[g0:g0 + 32, j, :]
                                    rhs = x0b[g0:g0 + 32,
                                              2 * i + dy:2 * i + dy + 2,
                                              dx:dx + 256]
                                nc.tensor.matmul(
                                    pss[r][:], lhsT, rhs,
                                    start=(t == 0), stop=(t == 11),
                                    tile_position=(g0, 0))
                        for r in range(4):
                            gt = gpool.tile([128, 2, 256], bf16, tag="gchunk")
                            nc.scalar.activation(
                                gt[:], pss[r][:], act_func, bias=btot[:, 0:1])
                            nc.gpsimd.tensor_mul(
                                mts[r][:, 2 * i:2 * i + 2, :], gt[:],
                                xbb[:, r, 2 * i:2 * i + 2, :])
                    for r in range(4):
                        nc.scalar.dma_start(
                            out_d[:, 64 * r + SB * bb: 64 * r + SB * bb + SB,
                                  :],
                            mts[r][:])

    nc.compile()
    _PROGRAM_CACHE[key] = nc
    return nc


def make_in_maps(x, w_dw, b_dw, w_f1, b_f1, w_f2, b_f2, w_f3, b_f3):
    import ml_dtypes
    bf = ml_dtypes.bfloat16
    x = np.asarray(x)
    B = x.shape[0]
    W_all, b_tot, wdwp = fold_weights(
        np.asarray(w_dw), np.asarray(b_dw), np.asarray(w_f1), np.asarray(b_f1),
        np.asarray(w_f2), np.asarray(b_f2), np.asarray(w_f3), np.asarray(b_f3))
    wall_b = np.ascontiguousarray(W_all.astype(bf))
    in_maps = [{"x": np.ascontiguousarray(x[i].astype(bf)),
                "wall": wall_b, "btot": b_tot, "wdwp": wdwp}
               for i in range(B)]
    return in_maps


def kernel(x, w_dw, b_dw, w_f1, b_f1, w_f2, b_f2, w_f3, b_f3):
    from concourse.bass_utils import run_bass_kernel_spmd

    x = np.asarray(x)
    B = x.shape[0]
    in_maps = make_in_maps(x, w_dw, b_dw, w_f1, b_f1, w_f2, b_f2, w_f3, b_f3)
    nc = build_program("Gelu")
    res = run_bass_kernel_spmd(nc, in_maps, list(range(B)))
    out = np.stack([res.results[i]["out"] for i in range(B)], axis=0)
    return out.astype(np.float32)
